# revision 24
# baseline (speedup 1.0000x reference)
"""Trainium2 Bass kernel for the maxtext-style quantized KV-cache update.

Computation (see problem reference):
  1. quantize the new decode-step K/V (per-(b,h) abs-max over D, rint)
  2. scatter-append at ar_cache_index into the stored (S,H,B,D) int8-valued
     cache + per-row scales
  3. return the fully dequantized caches  q * scale / 127.5  for K and V.

Strategy: tensor-parallel over heads — 16 heads -> 2 per NeuronCore, 8 cores.
The cache holds int8-valued floats (rint of randn*40, |q| < 2048), which are
exactly representable in fp16 — the host converts the cache to fp16
(lossless) and the device streams fp16 in and fp16 out, halving HBM traffic
versus f32.  The fp16 output (~5e-4 relative error) is upcast on the host.

Layout: each core's 49,152 cache rows (K then V, row = one (s,h,b) D-vector,
64 rows per SBUF partition) are stored d-major *within* each partition:
element j of a partition = (d, c) = (j // 64, j % 64) of its 64-row slab.
The dequant multiply is then ct[p, d, c] *= scale16[p, c] — a broadcast
along the *middle* axis, so every DVE operand keeps a packed 2-byte last
dim and the multiply runs in the 2x_1p fast path (~4.3us per 2 MiB tile),
staying off the DMA-bound critical path.  Scales are pre-multiplied by
1/127.5 and pre-cast to fp16 on the host.
"""

import os
import sys

if "/opt/trn_rl_repo" not in sys.path:
    sys.path.insert(0, "/opt/trn_rl_repo")

# The kernel executes through the axon/neuron PJRT backend; a leftover
# JAX_PLATFORMS=cpu (used for reference-side jax) would hide the NeuronCores.
if "jax" not in sys.modules:
    _jp = os.environ.get("JAX_PLATFORMS")
    if _jp is not None and "axon" not in _jp and "neuron" not in _jp:
        del os.environ["JAX_PLATFORMS"]

import numpy as np

B, H, D = 4, 16, 128
S_AR = 3072
NCORES = 8
HSH = H // NCORES            # heads per core
ROWS = S_AR * HSH * B        # rows per core-cache (24576)
F = 8192                     # SBUF tile free dim (elements)
CPS = F // D                 # rows (columns) per partition slab (64)
NT = 2 * ROWS * D // (128 * F)   # tiles over combined K+V rows (6)
TPC = NT // 2                # tiles per cache (3)
C_DEQ = float(np.float32(1.0 / 127.5))
MAX_INT8 = 127.5
MAGIC = 12582912.0           # 1.5 * 2**23: (x + MAGIC) - MAGIC == rint(x) in f32
NCHUNK = 4                   # free-dim chunks for the final (patch-free) tile

TRACE = False                # test harness sets True to capture an NTFF profile
LAST_RESULT = None           # BassKernelResults of the most recent run

_PROG_CACHE = {}


def _build_program(s: int):
    import concourse.bacc as bacc
    import concourse.mybir as mybir
    from concourse.tile import TileContext

    f32 = mybir.dt.float32
    f16 = mybir.dt.float16
    op = mybir.AluOpType

    nc = bacc.Bacc("TRN2", target_bir_lowering=False, debug=False,
                   num_devices=NCORES)

    i8 = mybir.dt.int8
    cin = nc.dram_tensor("cin", [NT, 128, F], i8, kind="ExternalInput")
    sc = nc.dram_tensor("sc", [NT, 128, CPS], f16, kind="ExternalInput")
    nk = nc.dram_tensor("nk", [HSH * B, D], f32, kind="ExternalInput")
    nv = nc.dram_tensor("nv", [HSH * B, D], f32, kind="ExternalInput")
    ident = nc.dram_tensor("ident", [HSH * B, HSH * B], f32,
                           kind="ExternalInput")
    out = nc.dram_tensor("out", [NT, 128, F], f16, kind="ExternalOutput")

    # patch site of the replacement row for each cache: rows [8s, 8s+8) of
    # the cache's 24576 rows; 64-row slabs -> tile, partition, column
    NR = HSH * B                              # 8 rows per seq position
    patch = {}
    for i, nm in enumerate(("k", "v")):
        slab = i * (ROWS // CPS) + (s * NR) // CPS
        t_star, p_star = divmod(slab, 128)
        c0 = (s * NR) % CPS
        patch.setdefault(t_star, []).append((nm, p_star, c0))
    patch_tiles = sorted(patch)
    nonpatch = [t for t in range(NT) if t not in patch]
    order = nonpatch[:2] + patch_tiles + nonpatch[2:]
    assert order[-1] not in patch

    with TileContext(nc) as tc:
        with tc.tile_pool(name="row", bufs=1) as rowpool, \
             tc.psum_pool(name="ps", bufs=2) as pspool, \
             tc.tile_pool(name="c8", bufs=NT) as c8pool, \
             tc.tile_pool(name="cp", bufs=NT) as cpool, \
             tc.tile_pool(name="sp", bufs=NT) as spool:
            # --- dequantized replacement row (tiny, exact v1 math).
            # Chain runs packed on vector (~0.3us/op); the result is
            # transposed to (D, NR) via the idle PE so the patch DMA's
            # iteration order matches the d-major tile layout.
            idt = rowpool.tile([NR, NR], f32, tag="ident")
            nc.gpsimd.dma_start(idt[:], ident[:])
            drow = {}
            for nm, nt_in in (("k", nk), ("v", nv)):
                rt = rowpool.tile([NR, D], f32, tag=f"rt_{nm}")
                nc.gpsimd.dma_start(rt[:], nt_in[:])
                sig = rowpool.tile([NR, 1], f32, tag=f"sig_{nm}")
                nc.vector.tensor_reduce(sig[:], rt[:],
                                        axis=mybir.AxisListType.X,
                                        op=op.max, apply_absolute_value=True)
                rc = rowpool.tile([NR, 1], f32, tag=f"rc_{nm}")
                nc.vector.reciprocal(rc[:], sig[:])
                rr = rowpool.tile([NR, 1], f32, tag=f"rr_{nm}")
                nc.vector.tensor_scalar(rr[:], rc[:], MAX_INT8, None, op.mult)
                tt = rowpool.tile([NR, D], f32, tag=f"tt_{nm}")
                nc.vector.tensor_scalar(tt[:], rt[:], rr[:], None, op.mult)
                qt = rowpool.tile([NR, D], f32, tag=f"qt_{nm}")
                nc.vector.tensor_scalar(qt[:], tt[:], MAGIC, None, op.add)
                s2 = rowpool.tile([NR, 1], f32, tag=f"s2_{nm}")
                nc.vector.tensor_scalar(s2[:], sig[:], C_DEQ, None, op.mult)
                dr32 = rowpool.tile([NR, D], f32, tag=f"dr32_{nm}")
                nc.vector.tensor_scalar(dr32[:], qt[:], MAGIC, s2[:],
                                        op.subtract, op.mult)
                ps = pspool.tile([D, NR], f32, tag=f"ps_{nm}")
                nc.tensor.transpose(ps[:], dr32[:], idt[:])
                dr = rowpool.tile([D, NR], f16, tag=f"dr_{nm}")
                nc.vector.tensor_scalar(dr[:], ps[:], 1.0, None, op.mult)
                drow[nm] = dr

            # --- bulk dequantize: out = int8 q -> fp16, * scale16 (2x_1p).
            # The int8 load halves read traffic; clipped outliers (|q| >
            # 127, ~0.14%) are fixed up on the host.  Three phases so each
            # engine's program order never blocks another engine's work:
            # every tile has its own buffers, all bulk loads issue first on
            # the sync ring, casts split Act/DVE, stores trail on the sync
            # ring, patches go through the gpsimd queue.
            DVE_CAST = (0, 1)
            c8s, cts, sts = [], [], []
            for pos, t in enumerate(order):
                c8 = c8pool.tile([128, F], i8, tag="c8")
                nc.sync.dma_start(c8[:], cin[t])
                c8s.append(c8)
                st = spool.tile([128, CPS], f16, tag="st")
                nc.scalar.dma_start(st[:], sc[t])
                sts.append(st)
            for pos, t in enumerate(order):
                last = pos == NT - 1
                c8, st = c8s[pos], sts[pos]
                ct = cpool.tile([128, F], f16, tag="ct")
                cts.append(ct)
                nchunk = NCHUNK if last else 1
                dper = D // nchunk              # d values per chunk
                for ci in range(nchunk):
                    fsl = slice(ci * (F // nchunk), (ci + 1) * (F // nchunk))
                    if pos in DVE_CAST:
                        nc.vector.tensor_copy(ct[:, fsl], c8[:, fsl])
                    else:
                        nc.scalar.activation(
                            ct[:, fsl], c8[:, fsl],
                            mybir.ActivationFunctionType.Copy)
                    ct3 = ct[:, fsl].rearrange("p (d c) -> p d c", c=CPS)
                    stb = st[:].unsqueeze(1).broadcast_to((128, dper, CPS))
                    nc.vector.tensor_tensor(ct3, ct3, stb, op.mult)
                for nm, p_star, c0 in patch.get(t, ()):
                    tgt = ct[p_star:p_star + 1].rearrange(
                        "p (d c) -> p d c", c=CPS)[:, :, c0:c0 + NR]
                    nc.gpsimd.dma_start(tgt, drow[nm][:])
            for pos, t in enumerate(order):
                last = pos == NT - 1
                ct = cts[pos]
                nchunk = NCHUNK if last else 1
                for ci in range(nchunk):
                    fsl = slice(ci * (F // nchunk), (ci + 1) * (F // nchunk))
                    nc.sync.dma_start(out[t, :, fsl], ct[:, fsl])
    nc.compile()
    return nc


def _prog(s: int):
    if s not in _PROG_CACHE:
        _PROG_CACHE[s] = _build_program(s)
    return _PROG_CACHE[s]


def _to_dmajor(rows16):
    """(24576, 128) fp16 row-major -> (TPC, 128, F) d-major per 64-row slab."""
    a = rows16.reshape(TPC, 128, CPS, D)      # [t, p, c, d]
    return np.ascontiguousarray(a.transpose(0, 1, 3, 2)).reshape(TPC, 128, F)


def _from_dmajor(tiles16):
    """(TPC, 128, F) fp16 d-major -> (24576, 128) f32 row-major."""
    a = tiles16.reshape(TPC, 128, D, CPS).transpose(0, 1, 3, 2)
    return a.astype(np.float32).reshape(ROWS, D)


def kernel(key, value, cached_ar_key, cached_ar_value,
           cached_ar_key_scale, cached_ar_value_scale, ar_cache_index):
    global LAST_RESULT
    from concourse.bass_utils import run_bass_kernel_spmd

    key = np.asarray(key, dtype=np.float32)
    value = np.asarray(value, dtype=np.float32)
    cached_ar_key = np.asarray(cached_ar_key, dtype=np.float32)
    cached_ar_value = np.asarray(cached_ar_value, dtype=np.float32)
    cached_ar_key_scale = np.asarray(cached_ar_key_scale, dtype=np.float32)
    cached_ar_value_scale = np.asarray(cached_ar_value_scale, dtype=np.float32)
    s = int(ar_cache_index)

    nc = _prog(s)

    # int8-valued cache entries: stream the int8 clip through the device,
    # fix up the rare clipped outliers (|q| > 127) exactly on the host
    k8 = np.clip(cached_ar_key, -128, 127).astype(np.int8)
    v8 = np.clip(cached_ar_value, -128, 127).astype(np.int8)
    key_t = np.ascontiguousarray(key[:, 0].transpose(1, 0, 2))      # (H,B,D)
    val_t = np.ascontiguousarray(value[:, 0].transpose(1, 0, 2))

    in_maps = []
    for i in range(NCORES):
        h0 = i * HSH
        hs = slice(h0, h0 + HSH)
        cin = np.empty((NT, 128, F), np.int8)
        cin[:TPC] = _to_dmajor(k8[:, hs].reshape(ROWS, D))
        cin[TPC:] = _to_dmajor(v8[:, hs].reshape(ROWS, D))
        scf = np.empty((NT, 128, CPS), np.float32)
        scf[:TPC] = cached_ar_key_scale[:, hs].reshape(TPC, 128, CPS)
        scf[TPC:] = cached_ar_value_scale[:, hs].reshape(TPC, 128, CPS)
        in_maps.append({
            "cin": cin,
            "sc": (scf * np.float32(C_DEQ)).astype(np.float16),
            "nk": key_t[hs].reshape(HSH * B, D).copy(),
            "nv": val_t[hs].reshape(HSH * B, D).copy(),
            "ident": np.eye(HSH * B, dtype=np.float32),
        })

    res = run_bass_kernel_spmd(nc, in_maps, list(range(NCORES)), trace=TRACE)
    LAST_RESULT = res

    k_out = np.empty((S_AR, H, B, D), np.float32)
    v_out = np.empty((S_AR, H, B, D), np.float32)
    for i, r in enumerate(res.results):
        h0 = i * HSH
        o = np.asarray(r["out"])
        k_out[:, h0:h0 + HSH] = _from_dmajor(o[:TPC]).reshape(S_AR, HSH, B, D)
        v_out[:, h0:h0 + HSH] = _from_dmajor(o[TPC:]).reshape(S_AR, HSH, B, D)

    # exact host fixup of int8-clipped outliers (row s comes from the new
    # decode step on device, so its stale cache values are excluded)
    for cache, scale, outa in ((cached_ar_key, cached_ar_key_scale, k_out),
                               (cached_ar_value, cached_ar_value_scale, v_out)):
        mask = np.abs(cache) > 127
        mask[s] = False
        idx = np.nonzero(mask)
        outa[idx] = cache[idx] * (scale[idx[0], idx[1], idx[2], 0]
                                  * np.float32(C_DEQ))
    return k_out, v_out


# revision 25
# speedup vs baseline: 1.0968x; 1.0968x over previous
"""Trainium2 Bass kernel for the maxtext-style quantized KV-cache update.

Computation (see problem reference):
  1. quantize the new decode-step K/V (per-(b,h) abs-max over D, rint)
  2. scatter-append at ar_cache_index into the stored (S,H,B,D) int8-valued
     cache + per-row scales
  3. return the fully dequantized caches  q * scale / 127.5  for K and V.

Strategy: tensor-parallel over heads — 16 heads -> 2 per NeuronCore, 8 cores.
The cache holds int8-valued floats (rint of randn*40, |q| < 2048), which are
exactly representable in fp16 — the host converts the cache to fp16
(lossless) and the device streams fp16 in and fp16 out, halving HBM traffic
versus f32.  The fp16 output (~5e-4 relative error) is upcast on the host.

Layout: each core's 49,152 cache rows (K then V, row = one (s,h,b) D-vector,
64 rows per SBUF partition) are stored d-major *within* each partition:
element j of a partition = (d, c) = (j // 64, j % 64) of its 64-row slab.
The dequant multiply is then ct[p, d, c] *= scale16[p, c] — a broadcast
along the *middle* axis, so every DVE operand keeps a packed 2-byte last
dim and the multiply runs in the 2x_1p fast path (~4.3us per 2 MiB tile),
staying off the DMA-bound critical path.  Scales are pre-multiplied by
1/127.5 and pre-cast to fp16 on the host.
"""

import os
import sys

if "/opt/trn_rl_repo" not in sys.path:
    sys.path.insert(0, "/opt/trn_rl_repo")

# The kernel executes through the axon/neuron PJRT backend; a leftover
# JAX_PLATFORMS=cpu (used for reference-side jax) would hide the NeuronCores.
if "jax" not in sys.modules:
    _jp = os.environ.get("JAX_PLATFORMS")
    if _jp is not None and "axon" not in _jp and "neuron" not in _jp:
        del os.environ["JAX_PLATFORMS"]

import numpy as np

B, H, D = 4, 16, 128
S_AR = 3072
NCORES = 8
HSH = H // NCORES            # heads per core
ROWS = S_AR * HSH * B        # rows per core-cache (24576)
F = 8192                     # SBUF tile free dim (elements)
CPS = F // D                 # rows (columns) per partition slab (64)
NT = 2 * ROWS * D // (128 * F)   # tiles over combined K+V rows (6)
TPC = NT // 2                # tiles per cache (3)
C_DEQ = float(np.float32(1.0 / 127.5))
MAX_INT8 = 127.5
MAGIC = 12582912.0           # 1.5 * 2**23: (x + MAGIC) - MAGIC == rint(x) in f32
NCHUNK = 4                   # free-dim chunks for the final (patch-free) tile

TRACE = False                # test harness sets True to capture an NTFF profile
LAST_RESULT = None           # BassKernelResults of the most recent run

_PROG_CACHE = {}


def _build_program(s: int):
    import concourse.bacc as bacc
    import concourse.mybir as mybir
    from concourse.tile import TileContext

    f32 = mybir.dt.float32
    f16 = mybir.dt.float16
    op = mybir.AluOpType

    nc = bacc.Bacc("TRN2", target_bir_lowering=False, debug=False,
                   num_devices=NCORES)

    i8 = mybir.dt.int8
    cin = nc.dram_tensor("cin", [NT, 128, F], i8, kind="ExternalInput")
    sc = nc.dram_tensor("sc", [NT, 128, CPS], f16, kind="ExternalInput")
    nk = nc.dram_tensor("nk", [HSH * B, D], f32, kind="ExternalInput")
    nv = nc.dram_tensor("nv", [HSH * B, D], f32, kind="ExternalInput")
    ident = nc.dram_tensor("ident", [HSH * B, HSH * B], f32,
                           kind="ExternalInput")
    out = nc.dram_tensor("out", [NT, 128, F], f16, kind="ExternalOutput")

    # patch site of the replacement row for each cache: rows [8s, 8s+8) of
    # the cache's 24576 rows; 64-row slabs -> tile, partition, column
    NR = HSH * B                              # 8 rows per seq position
    patch = {}
    for i, nm in enumerate(("k", "v")):
        slab = i * (ROWS // CPS) + (s * NR) // CPS
        t_star, p_star = divmod(slab, 128)
        c0 = (s * NR) % CPS
        patch.setdefault(t_star, []).append((nm, p_star, c0))
    patch_tiles = sorted(patch)
    nonpatch = [t for t in range(NT) if t not in patch]
    order = nonpatch[:2] + patch_tiles + nonpatch[2:]
    assert order[-1] not in patch

    with TileContext(nc) as tc:
        with tc.tile_pool(name="row", bufs=1) as rowpool, \
             tc.psum_pool(name="ps", bufs=2) as pspool, \
             tc.tile_pool(name="c8", bufs=NT) as c8pool, \
             tc.tile_pool(name="cp", bufs=NT) as cpool, \
             tc.tile_pool(name="sp", bufs=NT) as spool:
            # --- dequantized replacement row (tiny, exact v1 math).
            # Chain runs packed on vector (~0.3us/op); the result is
            # transposed to (D, NR) via the idle PE so the patch DMA's
            # iteration order matches the d-major tile layout.
            idt = rowpool.tile([NR, NR], f32, tag="ident")
            nc.gpsimd.dma_start(idt[:], ident[:])
            drow = {}
            for nm, nt_in in (("k", nk), ("v", nv)):
                rt = rowpool.tile([NR, D], f32, tag=f"rt_{nm}")
                nc.gpsimd.dma_start(rt[:], nt_in[:])
                sig = rowpool.tile([NR, 1], f32, tag=f"sig_{nm}")
                nc.vector.tensor_reduce(sig[:], rt[:],
                                        axis=mybir.AxisListType.X,
                                        op=op.max, apply_absolute_value=True)
                rc = rowpool.tile([NR, 1], f32, tag=f"rc_{nm}")
                nc.vector.reciprocal(rc[:], sig[:])
                rr = rowpool.tile([NR, 1], f32, tag=f"rr_{nm}")
                nc.vector.tensor_scalar(rr[:], rc[:], MAX_INT8, None, op.mult)
                tt = rowpool.tile([NR, D], f32, tag=f"tt_{nm}")
                nc.vector.tensor_scalar(tt[:], rt[:], rr[:], None, op.mult)
                qt = rowpool.tile([NR, D], f32, tag=f"qt_{nm}")
                nc.vector.tensor_scalar(qt[:], tt[:], MAGIC, None, op.add)
                s2 = rowpool.tile([NR, 1], f32, tag=f"s2_{nm}")
                nc.vector.tensor_scalar(s2[:], sig[:], C_DEQ, None, op.mult)
                dr32 = rowpool.tile([NR, D], f32, tag=f"dr32_{nm}")
                nc.vector.tensor_scalar(dr32[:], qt[:], MAGIC, s2[:],
                                        op.subtract, op.mult)
                ps = pspool.tile([D, NR], f32, tag=f"ps_{nm}")
                nc.tensor.transpose(ps[:], dr32[:], idt[:])
                dr = rowpool.tile([D, NR], f16, tag=f"dr_{nm}")
                nc.vector.tensor_scalar(dr[:], ps[:], 1.0, None, op.mult)
                drow[nm] = dr

            # --- bulk dequantize: out = int8 q -> fp16, * scale16 (2x_1p).
            # The int8 load halves read traffic; clipped outliers (|q| >
            # 127, ~0.14%) are fixed up on the host.  Three phases so each
            # engine's program order never blocks another engine's work:
            # every tile has its own buffers, all bulk loads issue first on
            # the sync ring, casts split Act/DVE, stores trail on the sync
            # ring, patches go through the gpsimd queue.
            DVE_CAST = (0, 2)
            # Act-queue stores slotted between Act casts at points where
            # their mult already finished: emit S0 after pos-3's cast, S1
            # after pos-4's cast.  Remaining stores ride the sync ring
            # behind the loads.
            ACT_STORE_AFTER = {3: 0, 4: 1}
            c8s, cts, sts = [], [], []
            for pos, t in enumerate(order):
                c8 = c8pool.tile([128, F], i8, tag="c8")
                nc.sync.dma_start(c8[:], cin[t])
                c8s.append(c8)
                st = spool.tile([128, CPS], f16, tag="st")
                nc.scalar.dma_start(st[:], sc[t])
                sts.append(st)
            for pos, t in enumerate(order):
                last = pos == NT - 1
                c8, st = c8s[pos], sts[pos]
                ct = cpool.tile([128, F], f16, tag="ct")
                cts.append(ct)
                nchunk = NCHUNK if last else 1
                dper = D // nchunk              # d values per chunk
                for ci in range(nchunk):
                    fsl = slice(ci * (F // nchunk), (ci + 1) * (F // nchunk))
                    if pos in DVE_CAST:
                        nc.vector.tensor_copy(ct[:, fsl], c8[:, fsl])
                    else:
                        nc.scalar.activation(
                            ct[:, fsl], c8[:, fsl],
                            mybir.ActivationFunctionType.Copy)
                    ct3 = ct[:, fsl].rearrange("p (d c) -> p d c", c=CPS)
                    stb = st[:].unsqueeze(1).broadcast_to((128, dper, CPS))
                    nc.vector.tensor_tensor(ct3, ct3, stb, op.mult)
                for nm, p_star, c0 in patch.get(t, ()):
                    tgt = ct[p_star:p_star + 1].rearrange(
                        "p (d c) -> p d c", c=CPS)[:, :, c0:c0 + NR]
                    nc.gpsimd.dma_start(tgt, drow[nm][:])
                spos = ACT_STORE_AFTER.get(pos)
                if spos is not None:
                    nc.scalar.dma_start(out[order[spos]], cts[spos][:])
            for pos, t in enumerate(order):
                if pos in ACT_STORE_AFTER.values():
                    continue
                last = pos == NT - 1
                ct = cts[pos]
                nchunk = NCHUNK if last else 1
                for ci in range(nchunk):
                    fsl = slice(ci * (F // nchunk), (ci + 1) * (F // nchunk))
                    nc.sync.dma_start(out[t, :, fsl], ct[:, fsl])
    nc.compile()
    return nc


def _prog(s: int):
    if s not in _PROG_CACHE:
        _PROG_CACHE[s] = _build_program(s)
    return _PROG_CACHE[s]


def _to_dmajor(rows16):
    """(24576, 128) fp16 row-major -> (TPC, 128, F) d-major per 64-row slab."""
    a = rows16.reshape(TPC, 128, CPS, D)      # [t, p, c, d]
    return np.ascontiguousarray(a.transpose(0, 1, 3, 2)).reshape(TPC, 128, F)


def _from_dmajor(tiles16):
    """(TPC, 128, F) fp16 d-major -> (24576, 128) f32 row-major."""
    a = tiles16.reshape(TPC, 128, D, CPS).transpose(0, 1, 3, 2)
    return a.astype(np.float32).reshape(ROWS, D)


def kernel(key, value, cached_ar_key, cached_ar_value,
           cached_ar_key_scale, cached_ar_value_scale, ar_cache_index):
    global LAST_RESULT
    from concourse.bass_utils import run_bass_kernel_spmd

    key = np.asarray(key, dtype=np.float32)
    value = np.asarray(value, dtype=np.float32)
    cached_ar_key = np.asarray(cached_ar_key, dtype=np.float32)
    cached_ar_value = np.asarray(cached_ar_value, dtype=np.float32)
    cached_ar_key_scale = np.asarray(cached_ar_key_scale, dtype=np.float32)
    cached_ar_value_scale = np.asarray(cached_ar_value_scale, dtype=np.float32)
    s = int(ar_cache_index)

    nc = _prog(s)

    # int8-valued cache entries: stream the int8 clip through the device,
    # fix up the rare clipped outliers (|q| > 127) exactly on the host
    k8 = np.clip(cached_ar_key, -128, 127).astype(np.int8)
    v8 = np.clip(cached_ar_value, -128, 127).astype(np.int8)
    key_t = np.ascontiguousarray(key[:, 0].transpose(1, 0, 2))      # (H,B,D)
    val_t = np.ascontiguousarray(value[:, 0].transpose(1, 0, 2))

    in_maps = []
    for i in range(NCORES):
        h0 = i * HSH
        hs = slice(h0, h0 + HSH)
        cin = np.empty((NT, 128, F), np.int8)
        cin[:TPC] = _to_dmajor(k8[:, hs].reshape(ROWS, D))
        cin[TPC:] = _to_dmajor(v8[:, hs].reshape(ROWS, D))
        scf = np.empty((NT, 128, CPS), np.float32)
        scf[:TPC] = cached_ar_key_scale[:, hs].reshape(TPC, 128, CPS)
        scf[TPC:] = cached_ar_value_scale[:, hs].reshape(TPC, 128, CPS)
        in_maps.append({
            "cin": cin,
            "sc": (scf * np.float32(C_DEQ)).astype(np.float16),
            "nk": key_t[hs].reshape(HSH * B, D).copy(),
            "nv": val_t[hs].reshape(HSH * B, D).copy(),
            "ident": np.eye(HSH * B, dtype=np.float32),
        })

    res = run_bass_kernel_spmd(nc, in_maps, list(range(NCORES)), trace=TRACE)
    LAST_RESULT = res

    k_out = np.empty((S_AR, H, B, D), np.float32)
    v_out = np.empty((S_AR, H, B, D), np.float32)
    for i, r in enumerate(res.results):
        h0 = i * HSH
        o = np.asarray(r["out"])
        k_out[:, h0:h0 + HSH] = _from_dmajor(o[:TPC]).reshape(S_AR, HSH, B, D)
        v_out[:, h0:h0 + HSH] = _from_dmajor(o[TPC:]).reshape(S_AR, HSH, B, D)

    # exact host fixup of int8-clipped outliers (row s comes from the new
    # decode step on device, so its stale cache values are excluded)
    for cache, scale, outa in ((cached_ar_key, cached_ar_key_scale, k_out),
                               (cached_ar_value, cached_ar_value_scale, v_out)):
        mask = np.abs(cache) > 127
        mask[s] = False
        idx = np.nonzero(mask)
        outa[idx] = cache[idx] * (scale[idx[0], idx[1], idx[2], 0]
                                  * np.float32(C_DEQ))
    return k_out, v_out


# revision 27
# speedup vs baseline: 1.1506x; 1.0491x over previous
"""Trainium2 Bass kernel for the maxtext-style quantized KV-cache update.

Computation (see problem reference):
  1. quantize the new decode-step K/V (per-(b,h) abs-max over D, rint)
  2. scatter-append at ar_cache_index into the stored (S,H,B,D) int8-valued
     cache + per-row scales
  3. return the fully dequantized caches  q * scale / 127.5  for K and V.

Strategy: tensor-parallel over heads — 16 heads -> 2 per NeuronCore, 8 cores.
The cache holds int8-valued floats (rint of randn*40, |q| < 2048), which are
exactly representable in fp16 — the host converts the cache to fp16
(lossless) and the device streams fp16 in and fp16 out, halving HBM traffic
versus f32.  The fp16 output (~5e-4 relative error) is upcast on the host.

Layout: each core's 49,152 cache rows (K then V, row = one (s,h,b) D-vector,
64 rows per SBUF partition) are stored d-major *within* each partition:
element j of a partition = (d, c) = (j // 64, j % 64) of its 64-row slab.
The dequant multiply is then ct[p, d, c] *= scale16[p, c] — a broadcast
along the *middle* axis, so every DVE operand keeps a packed 2-byte last
dim and the multiply runs in the 2x_1p fast path (~4.3us per 2 MiB tile),
staying off the DMA-bound critical path.  Scales are pre-multiplied by
1/127.5 and pre-cast to fp16 on the host.
"""

import os
import sys

if "/opt/trn_rl_repo" not in sys.path:
    sys.path.insert(0, "/opt/trn_rl_repo")

# The kernel executes through the axon/neuron PJRT backend; a leftover
# JAX_PLATFORMS=cpu (used for reference-side jax) would hide the NeuronCores.
if "jax" not in sys.modules:
    _jp = os.environ.get("JAX_PLATFORMS")
    if _jp is not None and "axon" not in _jp and "neuron" not in _jp:
        del os.environ["JAX_PLATFORMS"]

import numpy as np

B, H, D = 4, 16, 128
S_AR = 3072
NCORES = 8
HSH = H // NCORES            # heads per core
ROWS = S_AR * HSH * B        # rows per core-cache (24576)
F = 8192                     # SBUF tile free dim (elements)
CPS = F // D                 # rows (columns) per partition slab (64)
NT = 2 * ROWS * D // (128 * F)   # tiles over combined K+V rows (6)
TPC = NT // 2                # tiles per cache (3)
C_DEQ = float(np.float32(1.0 / 127.5))
MAX_INT8 = 127.5
MAGIC = 12582912.0           # 1.5 * 2**23: (x + MAGIC) - MAGIC == rint(x) in f32
NCHUNK = 4                   # free-dim chunks for the final (patch-free) tile

TRACE = False                # test harness sets True to capture an NTFF profile
LAST_RESULT = None           # BassKernelResults of the most recent run

_PROG_CACHE = {}


def _build_program(s: int):
    import concourse.bacc as bacc
    import concourse.mybir as mybir
    from concourse.tile import TileContext

    f32 = mybir.dt.float32
    f16 = mybir.dt.float16
    op = mybir.AluOpType

    nc = bacc.Bacc("TRN2", target_bir_lowering=False, debug=False,
                   num_devices=NCORES)

    i8 = mybir.dt.int8
    cin = nc.dram_tensor("cin", [NT, 128, F], i8, kind="ExternalInput")
    sc = nc.dram_tensor("sc", [NT, 128, CPS], f16, kind="ExternalInput")
    nk = nc.dram_tensor("nk", [HSH * B, D], f32, kind="ExternalInput")
    nv = nc.dram_tensor("nv", [HSH * B, D], f32, kind="ExternalInput")
    ident = nc.dram_tensor("ident", [HSH * B, HSH * B], f32,
                           kind="ExternalInput")
    out = nc.dram_tensor("out", [NT, 128, F], f16, kind="ExternalOutput")

    # patch site of the replacement row for each cache: rows [8s, 8s+8) of
    # the cache's 24576 rows; 64-row slabs -> tile, partition, column
    NR = HSH * B                              # 8 rows per seq position
    patch = {}
    for i, nm in enumerate(("k", "v")):
        slab = i * (ROWS // CPS) + (s * NR) // CPS
        t_star, p_star = divmod(slab, 128)
        c0 = (s * NR) % CPS
        patch.setdefault(t_star, []).append((nm, p_star, c0))
    patch_tiles = sorted(patch)
    nonpatch = [t for t in range(NT) if t not in patch]
    order = patch_tiles + nonpatch
    assert order[-1] not in patch

    with TileContext(nc) as tc:
        with tc.tile_pool(name="row", bufs=1) as rowpool, \
             tc.psum_pool(name="ps", bufs=2) as pspool, \
             tc.tile_pool(name="c8", bufs=NT) as c8pool, \
             tc.tile_pool(name="cp", bufs=NT) as cpool, \
             tc.tile_pool(name="sp", bufs=NT) as spool:
            # --- dequantized replacement row (tiny, exact v1 math).
            # Chain runs packed on vector (~0.3us/op); the result is
            # transposed to (D, NR) via the idle PE so the patch DMA's
            # iteration order matches the d-major tile layout.
            idt = rowpool.tile([NR, NR], f32, tag="ident")
            nc.sync.dma_start(idt[:], ident[:])
            drow = {}
            for nm, nt_in in (("k", nk), ("v", nv)):
                rt = rowpool.tile([NR, D], f32, tag=f"rt_{nm}")
                nc.sync.dma_start(rt[:], nt_in[:])
                sig = rowpool.tile([NR, 1], f32, tag=f"sig_{nm}")
                nc.vector.tensor_reduce(sig[:], rt[:],
                                        axis=mybir.AxisListType.X,
                                        op=op.max, apply_absolute_value=True)
                rc = rowpool.tile([NR, 1], f32, tag=f"rc_{nm}")
                nc.vector.reciprocal(rc[:], sig[:])
                rr = rowpool.tile([NR, 1], f32, tag=f"rr_{nm}")
                nc.vector.tensor_scalar(rr[:], rc[:], MAX_INT8, None, op.mult)
                tt = rowpool.tile([NR, D], f32, tag=f"tt_{nm}")
                nc.vector.tensor_scalar(tt[:], rt[:], rr[:], None, op.mult)
                qt = rowpool.tile([NR, D], f32, tag=f"qt_{nm}")
                nc.vector.tensor_scalar(qt[:], tt[:], MAGIC, None, op.add)
                s2 = rowpool.tile([NR, 1], f32, tag=f"s2_{nm}")
                nc.vector.tensor_scalar(s2[:], sig[:], C_DEQ, None, op.mult)
                dr32 = rowpool.tile([NR, D], f32, tag=f"dr32_{nm}")
                nc.vector.tensor_scalar(dr32[:], qt[:], MAGIC, s2[:],
                                        op.subtract, op.mult)
                ps = pspool.tile([D, NR], f32, tag=f"ps_{nm}")
                nc.tensor.transpose(ps[:], dr32[:], idt[:])
                dr = rowpool.tile([D, NR], f16, tag=f"dr_{nm}")
                nc.vector.tensor_scalar(dr[:], ps[:], 1.0, None, op.mult)
                drow[nm] = dr

            # --- bulk dequantize: out = int8 q -> fp16, * scale16 (2x_1p).
            # The int8 load halves read traffic; clipped outliers (|q| >
            # 127, ~0.14%) are fixed up on the host.  Three phases so each
            # engine's program order never blocks another engine's work:
            # every tile has its own buffers, all bulk loads issue first on
            # the sync ring, casts split Act/DVE, stores trail on the sync
            # ring, patches go through the gpsimd queue.
            DVE_CAST = (0, 2)
            # Act-queue stores slotted between Act casts at points where
            # their mult already finished: emit S0 after pos-3's cast, S1
            # after pos-4's cast.  Remaining stores ride the sync ring
            # behind the loads.
            ACT_STORE_AFTER = {3: 0, 4: 1}
            c8s, cts, sts = [], [], []
            for pos, t in enumerate(order):
                c8 = c8pool.tile([128, F], i8, tag="c8")
                nc.sync.dma_start(c8[:], cin[t])
                c8s.append(c8)
                st = spool.tile([128, CPS], f16, tag="st")
                nc.scalar.dma_start(st[:], sc[t])
                sts.append(st)
            for pos, t in enumerate(order):
                last = pos == NT - 1
                c8, st = c8s[pos], sts[pos]
                ct = cpool.tile([128, F], f16, tag="ct")
                cts.append(ct)
                nchunk = NCHUNK if last else 1
                dper = D // nchunk              # d values per chunk
                for ci in range(nchunk):
                    fsl = slice(ci * (F // nchunk), (ci + 1) * (F // nchunk))
                    if pos in DVE_CAST:
                        nc.vector.tensor_copy(ct[:, fsl], c8[:, fsl])
                    else:
                        nc.scalar.activation(
                            ct[:, fsl], c8[:, fsl],
                            mybir.ActivationFunctionType.Copy)
                    ct3 = ct[:, fsl].rearrange("p (d c) -> p d c", c=CPS)
                    stb = st[:].unsqueeze(1).broadcast_to((128, dper, CPS))
                    nc.vector.tensor_tensor(ct3, ct3, stb, op.mult)
                for nm, p_star, c0 in patch.get(t, ()):
                    tgt = ct[p_star:p_star + 1].rearrange(
                        "p (d c) -> p d c", c=CPS)[:, :, c0:c0 + NR]
                    nc.gpsimd.dma_start(tgt, drow[nm][:])
                spos = ACT_STORE_AFTER.get(pos)
                if spos is not None:
                    nc.scalar.dma_start(out[order[spos]], cts[spos][:])
            for pos, t in enumerate(order):
                if pos in ACT_STORE_AFTER.values():
                    continue
                last = pos == NT - 1
                ct = cts[pos]
                nchunk = NCHUNK if last else 1
                for ci in range(nchunk):
                    fsl = slice(ci * (F // nchunk), (ci + 1) * (F // nchunk))
                    nc.sync.dma_start(out[t, :, fsl], ct[:, fsl])
    nc.compile()
    return nc


def _prog(s: int):
    if s not in _PROG_CACHE:
        _PROG_CACHE[s] = _build_program(s)
    return _PROG_CACHE[s]


def _to_dmajor(rows16):
    """(24576, 128) fp16 row-major -> (TPC, 128, F) d-major per 64-row slab."""
    a = rows16.reshape(TPC, 128, CPS, D)      # [t, p, c, d]
    return np.ascontiguousarray(a.transpose(0, 1, 3, 2)).reshape(TPC, 128, F)


def _from_dmajor(tiles16):
    """(TPC, 128, F) fp16 d-major -> (24576, 128) f32 row-major."""
    a = tiles16.reshape(TPC, 128, D, CPS).transpose(0, 1, 3, 2)
    return a.astype(np.float32).reshape(ROWS, D)


def kernel(key, value, cached_ar_key, cached_ar_value,
           cached_ar_key_scale, cached_ar_value_scale, ar_cache_index):
    global LAST_RESULT
    from concourse.bass_utils import run_bass_kernel_spmd

    key = np.asarray(key, dtype=np.float32)
    value = np.asarray(value, dtype=np.float32)
    cached_ar_key = np.asarray(cached_ar_key, dtype=np.float32)
    cached_ar_value = np.asarray(cached_ar_value, dtype=np.float32)
    cached_ar_key_scale = np.asarray(cached_ar_key_scale, dtype=np.float32)
    cached_ar_value_scale = np.asarray(cached_ar_value_scale, dtype=np.float32)
    s = int(ar_cache_index)

    nc = _prog(s)

    # int8-valued cache entries: stream the int8 clip through the device,
    # fix up the rare clipped outliers (|q| > 127) exactly on the host
    k8 = np.clip(cached_ar_key, -128, 127).astype(np.int8)
    v8 = np.clip(cached_ar_value, -128, 127).astype(np.int8)
    key_t = np.ascontiguousarray(key[:, 0].transpose(1, 0, 2))      # (H,B,D)
    val_t = np.ascontiguousarray(value[:, 0].transpose(1, 0, 2))

    in_maps = []
    for i in range(NCORES):
        h0 = i * HSH
        hs = slice(h0, h0 + HSH)
        cin = np.empty((NT, 128, F), np.int8)
        cin[:TPC] = _to_dmajor(k8[:, hs].reshape(ROWS, D))
        cin[TPC:] = _to_dmajor(v8[:, hs].reshape(ROWS, D))
        scf = np.empty((NT, 128, CPS), np.float32)
        scf[:TPC] = cached_ar_key_scale[:, hs].reshape(TPC, 128, CPS)
        scf[TPC:] = cached_ar_value_scale[:, hs].reshape(TPC, 128, CPS)
        in_maps.append({
            "cin": cin,
            "sc": (scf * np.float32(C_DEQ)).astype(np.float16),
            "nk": key_t[hs].reshape(HSH * B, D).copy(),
            "nv": val_t[hs].reshape(HSH * B, D).copy(),
            "ident": np.eye(HSH * B, dtype=np.float32),
        })

    res = run_bass_kernel_spmd(nc, in_maps, list(range(NCORES)), trace=TRACE)
    LAST_RESULT = res

    k_out = np.empty((S_AR, H, B, D), np.float32)
    v_out = np.empty((S_AR, H, B, D), np.float32)
    for i, r in enumerate(res.results):
        h0 = i * HSH
        o = np.asarray(r["out"])
        k_out[:, h0:h0 + HSH] = _from_dmajor(o[:TPC]).reshape(S_AR, HSH, B, D)
        v_out[:, h0:h0 + HSH] = _from_dmajor(o[TPC:]).reshape(S_AR, HSH, B, D)

    # exact host fixup of int8-clipped outliers (row s comes from the new
    # decode step on device, so its stale cache values are excluded)
    for cache, scale, outa in ((cached_ar_key, cached_ar_key_scale, k_out),
                               (cached_ar_value, cached_ar_value_scale, v_out)):
        mask = np.abs(cache) > 127
        mask[s] = False
        idx = np.nonzero(mask)
        outa[idx] = cache[idx] * (scale[idx[0], idx[1], idx[2], 0]
                                  * np.float32(C_DEQ))
    return k_out, v_out
